# revision 4
# baseline (speedup 1.0000x reference)
"""Trainium2 Bass kernel for causal self-attention with GQA + RoPE.

Problem: x[2,2048,2048], Wq[2048,2048], Wkv[2048,1024], Wproj[2048,2048],
16 q heads, 4 kv heads, head_dim 128, causal softmax, RoPE.

Sharding: 8 cores <-> (batch b in {0,1}) x (kv group g in {0..3}).
Each core computes its 4 q heads + 1 kv head for one batch, producing a
partial output z_partial[T, C] = y_heads @ Wproj[rows of those heads].
Host sums the 4 partials per batch (the Wproj row-shard allreduce).

v2 layout: everything bf16 on the wire and in SBUF (PSUM accum stays f32).
Fused per-i-tile pipeline: proj(it) -> rope -> attention(h=0..3, it) ->
out-proj(it), so the Tile scheduler fills exp(ACT)-bound attention windows
with projection / out-proj PE work. Causal mask is folded into the scores
PSUM via an identity-matmul of a -1e30 triangle constant. V is transposed
to [j, d] layout with DMA-transpose. RoPE elementwise runs on gpsimd
(mul/add) + DVE/ACT; softmax normalizer Z accumulates via ones-matmul of
DVE pair-summed exp tiles.
"""

import sys

for _p in ("/opt/trn_rl_repo",):
    if _p not in sys.path:
        sys.path.insert(0, _p)

import numpy as np

B, T, C = 2, 2048, 2048
NH, NKV, HD = 16, 4, 128
GH = NH // NKV  # q heads per core = 4
GW = GH * HD  # 512
NCC = C // 128  # 16 contraction chunks
NIT = T // 512  # 4 i-tiles
NCORES = 8
NEG = -1.0e30

_CACHE = {}


def _host_tables():
    if "tables" in _CACHE:
        return _CACHE["tables"]
    import ml_dtypes

    bf16 = ml_dtypes.bfloat16
    m = np.arange(HD // 2)
    theta = 10000.0 ** (-2.0 * m / HD)
    fr = np.outer(np.arange(T, dtype=np.float64), theta)  # [T, 64]
    cos = np.cos(fr)
    sin = np.sin(fr)
    cosT = np.ascontiguousarray(np.concatenate([cos, cos], 1).T).astype(bf16)
    sinT = np.ascontiguousarray(np.concatenate([sin, sin], 1).T).astype(bf16)
    rotm = np.zeros((HD, HD), dtype=np.float32)
    for mm in range(HD // 2):
        rotm[mm + 64, mm] = -1.0  # out[m] = -x[m+64], m < 64
        rotm[mm, mm + 64] = 1.0  # out[m] = x[m-64],  m >= 64
    ident = np.eye(128, dtype=np.float32).astype(bf16)
    ones = np.ones((128, 128), dtype=np.float32).astype(bf16)
    # mtri[j, i] = 0 if i >= j else NEG  (strip-local causal triangle)
    jl = np.arange(128)[:, None]
    il = np.arange(128)[None, :]
    mtri = np.where(il >= jl, 0.0, NEG).astype(np.float32).astype(bf16)
    _CACHE["tables"] = (cosT, sinT, rotm.astype(bf16), ident, ones, mtri)
    return _CACHE["tables"]


def _build_nc():
    if "nc" in _CACHE:
        return _CACHE["nc"]
    import concourse.bacc as bacc
    import concourse.mybir as mybir
    import concourse.tile as tile

    f32 = mybir.dt.float32
    bf16 = mybir.dt.bfloat16
    Exp = mybir.ActivationFunctionType.Exp

    nc = bacc.Bacc("TRN2", debug=False, num_devices=NCORES)

    def din(name, shape, dt=bf16):
        return nc.dram_tensor(name, shape, dt, kind="ExternalInput").ap()

    xT = din("xT", [C, T])
    wq = din("wq", [C, GW])
    wk = din("wk", [C, HD])
    wv = din("wv", [C, HD])
    wp = din("wp", [GW, C])
    cosT = din("cosT", [HD, T])
    sinT = din("sinT", [HD, T])
    rotm = din("rotm", [HD, HD])
    identm = din("identm", [128, 128])
    onesm = din("onesm", [128, 128])
    mtrim = din("mtrim", [128, 128])
    z = nc.dram_tensor("z", [T, C], bf16, kind="ExternalOutput").ap()

    xT_r = xT.rearrange("(co p) t -> p co t", p=128)
    wq_r = wq.rearrange("(co p) d -> p co d", p=128)
    wk_r = wk.rearrange("(co p) d -> p co d", p=128)
    wv_r = wv.rearrange("(co p) d -> p co d", p=128)
    wp_r = wp.rearrange("(hc p) c -> p hc c", p=128)

    with tile.TileContext(nc) as tc:
        with (
            tc.tile_pool(name="persist", bufs=1) as persist,
            tc.tile_pool(name="xf", bufs=4) as xf_pool,
            tc.tile_pool(name="ep", bufs=6) as e_pool,
            tc.tile_pool(name="es", bufs=3) as es_pool,
            tc.tile_pool(name="rp", bufs=3) as r_pool,
            tc.tile_pool(name="nrm", bufs=2) as n_pool,
            tc.tile_pool(name="zr", bufs=2) as z_pool,
            tc.tile_pool(name="pss", bufs=3, space="PSUM") as pss_pool,
            tc.tile_pool(name="pyz", bufs=1, space="PSUM") as pyz_pool,
            tc.tile_pool(name="mini", bufs=3, space="PSUM") as mini_pool,
        ):
            qT = [
                persist.tile([128, T], bf16, tag=f"qT{h}", name=f"qT{h}")
                for h in range(GH)
            ]
            kT = persist.tile([128, T], bf16, tag="kT", name="kT")
            vv = persist.tile([128, T], bf16, tag="vv", name="vv")
            yT = [
                persist.tile([128, T], bf16, tag=f"yT{h}", name=f"yT{h}")
                for h in range(GH)
            ]
            cos_t = persist.tile([128, T], bf16, tag="cos", name="cos")
            sin_t = persist.tile([128, T], bf16, tag="sin", name="sin")
            rotm_t = persist.tile([128, 128], bf16, tag="rotm", name="rotm")
            ident_t = persist.tile([128, 128], bf16, tag="ident", name="ident")
            ones_t = persist.tile([128, 128], bf16, tag="ones", name="ones")
            mtri_t = persist.tile([128, 128], bf16, tag="mtri", name="mtri")
            wq_t = persist.tile([128, NCC, GW], bf16, tag="wq", name="wq")
            wk_t = persist.tile([128, NCC, HD], bf16, tag="wk", name="wk")
            wv_t = persist.tile([128, NCC, HD], bf16, tag="wv", name="wv")
            wp_t = persist.tile([128, GH, C], bf16, tag="wp", name="wp")

            # ---- preamble DMAs, spread across queues ----
            # scalar queue: weights (k/v first: they gate the first pass)
            nc.scalar.dma_start(wk_t[:], wk_r)
            nc.scalar.dma_start(wv_t[:], wv_r)
            nc.scalar.dma_start(wq_t[:], wq_r)
            nc.scalar.dma_start(wp_t[:], wp_r)
            # gpsimd queue: small constants first, then rope tables
            nc.gpsimd.dma_start(rotm_t[:], rotm)
            nc.gpsimd.dma_start(ident_t[:], identm)
            nc.gpsimd.dma_start(ones_t[:], onesm)
            nc.gpsimd.dma_start(mtri_t[:], mtrim)
            nc.gpsimd.dma_start(cos_t[:], cosT)
            nc.gpsimd.dma_start(sin_t[:], sinT)

            # x i-tile halves: [128, 8, 512] each, two per i-tile
            def xf_load(it):
                I0 = it * 512
                halves = []
                for hh in range(2):
                    t = xf_pool.tile([128, 8, 512], bf16, tag="xf", name="xf")
                    nc.sync.dma_start(
                        t[:], xT_r[:, hh * 8 : (hh + 1) * 8, I0 : I0 + 512]
                    )
                    halves.append(t)
                return halves

            xf_next = xf_load(0)

            def proj_pass(xf, wsl):
                # accumulate one [128, 512] output over the 16 c-chunks
                acc = mini_pool.tile([128, 512], f32, tag="mini", name="pacc")
                for c in range(NCC):
                    nc.tensor.matmul(
                        acc[:],
                        wsl(c),
                        xf[c // 8][:, c % 8, :],
                        start=(c == 0),
                        stop=(c == NCC - 1),
                    )
                return acc

            def rope(acc, dst, I0):
                plain = r_pool.tile([128, 512], bf16, tag="plain", name="plain")
                nc.any.tensor_copy(out=plain[:], in_=acc[:])
                psr = mini_pool.tile([128, 512], f32, tag="mini", name="psr")
                nc.tensor.matmul(psr[:], rotm_t[:], plain[:], start=True, stop=True)
                t1 = r_pool.tile([128, 512], bf16, tag="t1", name="t1")
                nc.gpsimd.tensor_mul(out=t1[:], in0=plain[:], in1=cos_t[:, I0 : I0 + 512])
                t2 = r_pool.tile([128, 512], bf16, tag="t2", name="t2")
                nc.any.tensor_mul(out=t2[:], in0=psr[:], in1=sin_t[:, I0 : I0 + 512])
                nc.gpsimd.tensor_add(out=dst[:, I0 : I0 + 512], in0=t1[:], in1=t2[:])

            for it in range(NIT):
                I0 = it * 512
                xf = xf_next
                if it + 1 < NIT:
                    xf_next = xf_load(it + 1)

                # ---- P(it): projections + rope + v transpose ----
                acc_k = proj_pass(xf, lambda c: wk_t[:, c])
                rope(acc_k, kT, I0)
                acc_v = proj_pass(xf, lambda c: wv_t[:, c])
                vstage = r_pool.tile([128, 512], bf16, tag="vst", name="vst")
                nc.any.tensor_copy(out=vstage[:], in_=acc_v[:])
                for q in range(4):
                    jc = 4 * it + q
                    nc.scalar.dma_start(
                        vv[:, jc * 128 : (jc + 1) * 128],
                        vstage[:, q * 128 : (q + 1) * 128],
                        transpose=True,
                    )
                for h in range(GH):
                    acc_q = proj_pass(
                        xf, lambda c, h=h: wq_t[:, c, h * HD : (h + 1) * HD]
                    )
                    rope(acc_q, qT[h], I0)

                # ---- A(h, it): attention ----
                nj = 4 * (it + 1)
                for h in range(GH):
                    ps_y = pyz_pool.tile([128, 512], f32, tag="psy", name="psy")
                    ps_z = pyz_pool.tile([128, 512], f32, tag="psz", name="psz")
                    prev = None
                    for jc in range(nj):
                        diag = jc >= 4 * it
                        w0 = (jc - 4 * it) * 128 if diag else 0
                        w = 512 - w0
                        ps_s = pss_pool.tile([128, 512], f32, tag="pss", name="pss")
                        nc.tensor.matmul(
                            ps_s[:, :w],
                            kT[:, jc * 128 : (jc + 1) * 128],
                            qT[h][:, I0 + w0 : I0 + 512],
                            start=True,
                            stop=not diag,
                        )
                        if diag:
                            # fold the strip-local causal triangle into PSUM
                            nc.tensor.matmul(
                                ps_s[:, :128],
                                ident_t[:],
                                mtri_t[:],
                                start=False,
                                stop=True,
                            )
                        e = e_pool.tile([128, 512], bf16, tag="e", name="e")
                        nc.scalar.activation(e[:, :w], ps_s[:, :w], Exp)
                        nc.tensor.matmul(
                            ps_y[:, w0:512],
                            vv[:, jc * 128 : (jc + 1) * 128],
                            e[:, :w],
                            start=(jc == 0),
                            stop=(jc == nj - 1),
                        )
                        if jc % 2 == 0:
                            prev = (e, w, w0)
                        else:
                            pe_, pw, pw0 = prev
                            esum = es_pool.tile(
                                [128, 512], bf16, tag="es", name="es"
                            )
                            if w < pw:
                                nc.any.tensor_copy(
                                    out=esum[:, 0:128], in_=pe_[:, 0:128]
                                )
                                nc.any.tensor_add(
                                    out=esum[:, 128:pw],
                                    in0=pe_[:, 128:pw],
                                    in1=e[:, :w],
                                )
                            else:
                                nc.any.tensor_add(
                                    out=esum[:], in0=pe_[:], in1=e[:]
                                )
                            nc.tensor.matmul(
                                ps_z[:, pw0:512],
                                ones_t[:],
                                esum[:, :pw],
                                start=(jc == 1),
                                stop=(jc == nj - 1),
                            )
                    rz = n_pool.tile([128, 512], f32, tag="rz", name="rz")
                    nc.vector.reciprocal_approx_fast(out=rz[:], in_=ps_z[:])
                    nc.any.tensor_mul(
                        out=yT[h][:, I0 : I0 + 512], in0=ps_y[:], in1=rz[:]
                    )

                # ---- O(it): output projection for this i-tile's 4 row chunks ----
                for ic in range(4 * it, 4 * (it + 1)):
                    zr = z_pool.tile([128, C], bf16, tag="zr", name="zr")
                    for ct in range(C // 512):
                        po = mini_pool.tile([128, 512], f32, tag="mini", name="po")
                        for hc in range(GH):
                            nc.tensor.matmul(
                                po[:],
                                yT[hc][:, ic * 128 : (ic + 1) * 128],
                                wp_t[:, hc, ct * 512 : (ct + 1) * 512],
                                start=(hc == 0),
                                stop=(hc == GH - 1),
                            )
                        nc.any.tensor_copy(
                            out=zr[:, ct * 512 : (ct + 1) * 512], in_=po[:]
                        )
                    nc.sync.dma_start(z[ic * 128 : (ic + 1) * 128, :], zr[:])

    nc.compile()
    _CACHE["nc"] = nc
    return nc


def _in_maps(x, Wq, Wkv, Wproj):
    import ml_dtypes

    bf16 = ml_dtypes.bfloat16
    cosT, sinT, rotm, ident, ones, mtri = _host_tables()
    s = np.float32(1.0 / np.sqrt(HD))
    xTs = [np.ascontiguousarray(x[b].T).astype(bf16) for b in range(B)]
    maps = []
    for core in range(NCORES):
        b, g = divmod(core, NKV)
        maps.append(
            {
                "xT": xTs[b],
                "wq": np.ascontiguousarray(Wq[:, g * GW : (g + 1) * GW] * s).astype(
                    bf16
                ),
                "wk": np.ascontiguousarray(Wkv[:, g * HD : (g + 1) * HD]).astype(bf16),
                "wv": np.ascontiguousarray(
                    Wkv[:, NKV * HD + g * HD : NKV * HD + (g + 1) * HD]
                ).astype(bf16),
                "wp": np.ascontiguousarray(Wproj[g * GW : (g + 1) * GW, :]).astype(
                    bf16
                ),
                "cosT": cosT,
                "sinT": sinT,
                "rotm": rotm,
                "identm": ident,
                "onesm": ones,
                "mtrim": mtri,
            }
        )
    return maps


def _run(inputs, trace=False, trace_kwargs=None):
    from concourse.bass_utils import run_bass_kernel_spmd

    nc = _build_nc()
    maps = _in_maps(
        np.asarray(inputs["x"], dtype=np.float32),
        np.asarray(inputs["Wq"], dtype=np.float32),
        np.asarray(inputs["Wkv"], dtype=np.float32),
        np.asarray(inputs["Wproj"], dtype=np.float32),
    )
    res = run_bass_kernel_spmd(
        nc, maps, list(range(NCORES)), trace=trace, **(trace_kwargs or {})
    )
    out = np.zeros((B, T, C), dtype=np.float32)
    for core in range(NCORES):
        b = core // NKV
        out[b] += np.asarray(res.results[core]["z"]).astype(np.float32)
    return out, res


def kernel(x, Wq, Wkv, Wproj):
    out, _ = _run({"x": x, "Wq": Wq, "Wkv": Wkv, "Wproj": Wproj}, trace=False)
    return out
